# revision 1
# baseline (speedup 1.0000x reference)
"""DGL capsule routing layer on 8 trn2 NeuronCores (Bass/Tile).

Math: for routing_num iterations,
    c = softmax(b, axis=out)                        # b0 = 0
    s = einsum('io,iof->of', c, uh)
    v = squash(s)
    b = b + einsum('iof,of->io', uh, v)
Output: final v [OUT, F].

Key identity: b_t = uh . (v_1 + ... + v_t)  (b is linear in uh), so b is
never materialized across iterations; each iteration is one streaming pass
over uh with w_t = cumulative sum of v's:
    pass t: b = sum_f uh[i,o,f]*w[o,f]; e = exp(b); r_i = 1/sum_o e
            s[o,f] = sum_i r_i * e[i,o] * uh[i,o,f]   (partial per core)
            AllReduce(s); v = squash(s); w += v
Pass 1 has c uniform (=1/OUT) so it is a pure PE pass.

Sharding: i (in_nodes) split across 8 cores, 512 rows each (4 blocks of
128 partitions). Engine plan per 2048-wide o-f chunk (passes >= 2):
  GpSimd: tm = uh * w_bcast        (2-input mul; DVE TT never contends)
  DVE:    b-slice = segsum_f(tm);  p = e * uh (e broadcast over f)
  ACT:    e = exp(b) with fused denominator accum; psum flushes
  PE:     s-partial = sum_i rinv[i]*p[i,:] as 4x N=512 matmuls with
          rinv as the 1-column stationary operand -> psum [1,2048]
The per-block s partials go straight to DRAM [4,16384]; the AllReduce sums
over cores, and the cheap cross-block sum happens after the AR in the
partition-spread [128,128] layout (3 DVE adds).
"""

import numpy as np
from contextlib import ExitStack

import concourse.bass as bass
import concourse.mybir as mybir
import concourse.tile as tile
from concourse import bacc
from concourse import bass_utils

F32 = mybir.dt.float32
AX = mybir.AxisListType
AF = mybir.ActivationFunctionType

IN_NODES, OUT_NODES, F_SIZE = 4096, 1024, 16
CORES = 8
I_LOC = IN_NODES // CORES          # 512 in-nodes per core
ROW = OUT_NODES * F_SIZE           # 16384 floats per in-node row
P = 128
NBLK = I_LOC // P                  # 4 i-blocks per core
QT = 4096                          # streamed quarter width (elems/partition)
NQT = ROW // QT                    # 4 quarters per block
CH = 2048                          # chunk/piece width (elems/partition)
NCH_Q = QT // CH                   # 2 chunks per quarter
NMM = CH // 512                    # 4 matmuls per piece
F32R_MM = True                     # fast-path fp32 matmuls (1 cyc/row)
MM_DT = mybir.dt.float32r if F32R_MM else F32


def _body(nc, tc, uh, v_out, R, rg):
    uh_t = uh.rearrange("(n p) r -> n p r", p=P)   # [NBLK, 128, 16384]

    with ExitStack() as ctx:
        io = ctx.enter_context(tc.tile_pool(name="io", bufs=4))
        work = ctx.enter_context(tc.tile_pool(name="work", bufs=4))
        small = ctx.enter_context(tc.tile_pool(name="small", bufs=2))
        persist = ctx.enter_context(tc.tile_pool(name="persist", bufs=1))
        pspool = ctx.enter_context(tc.tile_pool(name="pspool", bufs=2, space="PSUM"))
        dram = ctx.enter_context(tc.tile_pool(name="dram", bufs=2, space="DRAM"))

        c0_f = persist.tile([P, 1], F32, name="c0_f")
        nc.vector.memset(c0_f, 1.0 / OUT_NODES)
        c0 = persist.tile([P, 1], MM_DT, name="c0")
        nc.vector.tensor_copy(c0, c0_f)
        w_sb = w_acc = None
        if R > 1:
            w_sb = persist.tile([P, ROW], F32, name="w_sb")
            w_acc = persist.tile([P, P], F32, name="w_acc")

        for t in range(1, R + 1):
            ar_in = dram.tile([NBLK, ROW], F32, tag="ar_in")
            for blk in range(NBLK):
                uts = []
                for q in range(NQT):
                    ut = io.tile([P, QT], F32, tag="ut")
                    nc.sync.dma_start(ut, uh_t[blk, :, q * QT:(q + 1) * QT])
                    uts.append(ut)
                if t == 1:
                    rinv = c0
                else:
                    b = small.tile([P, OUT_NODES], F32, tag="b")
                    for q in range(NQT):
                        for k in range(NCH_Q):
                            sl = slice(k * CH, (k + 1) * CH)
                            g0 = q * QT + k * CH
                            tm = work.tile([P, CH], F32, tag="tm")
                            # b-mul on GpSimd (concurrent with DVE TT/reduce)
                            nc.gpsimd.tensor_mul(
                                tm, uts[q][:, sl], w_sb[:, g0:g0 + CH])
                            o0 = g0 // F_SIZE
                            nc.vector.reduce_sum(
                                b[:, o0:o0 + CH // F_SIZE],
                                tm.rearrange("p (o f) -> p o f", f=F_SIZE),
                                axis=AX.X,
                            )
                    e = small.tile([P, OUT_NODES], F32, tag="e")
                    den = small.tile([P, 1], F32, tag="den")
                    nc.scalar.activation(e, b, AF.Exp, accum_out=den)
                    rinv_f = small.tile([P, 1], F32, tag="rinv_f")
                    nc.vector.reciprocal(rinv_f, den)
                    rinv = small.tile([P, 1], MM_DT, tag="rinv")
                    nc.vector.tensor_copy(rinv, rinv_f)
                for q in range(NQT):
                    for k in range(NCH_Q):
                        sl = slice(k * CH, (k + 1) * CH)
                        g0 = q * QT + k * CH
                        pt = work.tile([P, CH], MM_DT, tag="tm")
                        if t == 1:
                            # round to f32r on idle DVE (pass 1 only)
                            nc.vector.tensor_copy(pt, uts[q][:, sl])
                        else:
                            o0 = g0 // F_SIZE
                            och = CH // F_SIZE
                            nc.vector.tensor_mul(
                                pt.rearrange("p (o f) -> p o f", f=F_SIZE),
                                uts[q][:, sl].rearrange(
                                    "p (o f) -> p o f", f=F_SIZE),
                                e[:, o0:o0 + och][:, :, None].broadcast_to(
                                    [P, och, F_SIZE]),
                            )
                        ps = pspool.tile([1, CH], F32, tag="ps")
                        for c in range(NMM):
                            nc.tensor.matmul(
                                ps[:, c * 512:(c + 1) * 512],
                                rinv,
                                pt[:, c * 512:(c + 1) * 512],
                                start=True, stop=True,
                                skip_group_check=True,
                            )
                        fl = small.tile([1, CH], F32, tag="fl")
                        nc.scalar.copy(fl, ps)
                        nc.sync.dma_start(ar_in[blk, g0:g0 + CH], fl)
            ar_out = dram.tile([NBLK, ROW], F32, tag="ar_out")
            nc.gpsimd.collective_compute(
                "AllReduce", mybir.AluOpType.add, replica_groups=rg,
                ins=[ar_in.opt()], outs=[ar_out.opt()],
            )
            # s2[p,(j,f)] with o = p*8+j: sum the 4 block rows post-AR
            slds = []
            for blk in range(NBLK):
                sld = small.tile([P, P], F32, tag="sld", bufs=4)
                nc.sync.dma_start(
                    sld, ar_out[blk].rearrange("(p q) -> p q", p=P))
                slds.append(sld)
            s2 = small.tile([P, P], F32, tag="s2")
            nc.vector.tensor_add(s2, slds[0], slds[1])
            nc.vector.tensor_add(s2, s2, slds[2])
            nc.vector.tensor_add(s2, s2, slds[3])
            # squash: v = s * sqrt(sq)/(1+sq), sq = sum_f s^2
            ssq = small.tile([P, P], F32, tag="ssq")
            nc.vector.tensor_mul(ssq, s2, s2)
            sq = small.tile([P, 8], F32, tag="sq")
            nc.vector.reduce_sum(
                sq, ssq.rearrange("p (j f) -> p j f", f=F_SIZE), axis=AX.X)
            # sqrt via exp(0.5*ln(x)): stays in the exp/ln ACT table set
            lnq = small.tile([P, 8], F32, tag="lnq")
            nc.scalar.activation(lnq, sq, AF.Ln)
            y = small.tile([P, 8], F32, tag="y")
            nc.scalar.activation(y, lnq, AF.Exp, scale=0.5)
            # one Newton step: y <- 0.5*(y + sq/y)
            ry = small.tile([P, 8], F32, tag="ry")
            nc.vector.reciprocal(ry, y)
            t1 = small.tile([P, 8], F32, tag="t1")
            nc.vector.tensor_mul(t1, sq, ry)
            nc.vector.tensor_add(t1, t1, y)
            nc.vector.tensor_scalar_mul(t1, t1, 0.5)
            d1 = small.tile([P, 8], F32, tag="d1")
            nc.vector.tensor_scalar_add(d1, sq, 1.0)
            rd = small.tile([P, 8], F32, tag="rd")
            nc.vector.reciprocal(rd, d1)
            sc = small.tile([P, 8], F32, tag="sc")
            nc.vector.tensor_mul(sc, t1, rd)
            v_sb = small.tile([P, P], F32, tag="v_sb")
            nc.vector.tensor_mul(
                v_sb.rearrange("p (j f) -> p j f", f=F_SIZE),
                s2.rearrange("p (j f) -> p j f", f=F_SIZE),
                sc[:, :, None].broadcast_to([P, 8, F_SIZE]),
            )
            if t == R:
                nc.sync.dma_start(
                    v_out.rearrange("(p j) f -> p (j f)", j=8), v_sb)
            else:
                if t == 1:
                    nc.scalar.copy(w_acc, v_sb)
                else:
                    nc.vector.tensor_add(w_acc, w_acc, v_sb)
                # broadcast w to all partitions via DRAM round-trip:
                # w_acc[p,(j,f)] -> flat w_dram[o*16+f] -> [128, ROW] bcast
                w_dram = dram.tile([ROW], F32, tag="w_dram")
                nc.sync.dma_start(
                    w_dram.rearrange("(p q) -> p q", p=P), w_acc)
                wd_b = w_dram.unsqueeze(0)
                for j in range(8):
                    sl = slice(j * CH, (j + 1) * CH)
                    nc.sync.dma_start(
                        w_sb[:, sl],
                        wd_b[:, sl].broadcast_to([P, CH]))


def _build(routing_num: int):
    R = int(routing_num)
    assert R >= 1
    nc = bacc.Bacc(
        "TRN2", target_bir_lowering=False, debug=False, num_devices=CORES)
    uh = nc.dram_tensor("uh", [I_LOC, ROW], F32, kind="ExternalInput")
    v_out = nc.dram_tensor("v_out", [OUT_NODES, F_SIZE], F32,
                           kind="ExternalOutput")
    rg = [list(range(CORES))]
    with tile.TileContext(nc) as tc:
        _body(nc, tc, uh.ap(), v_out.ap(), R, rg)
    nc.compile()
    return nc


_CACHE: dict = {}


def _get_nc(routing_num: int):
    R = int(routing_num)
    if R not in _CACHE:
        _CACHE[R] = _build(R)
    return _CACHE[R]


def _shard(u_hat: np.ndarray):
    uh = np.ascontiguousarray(np.asarray(u_hat, dtype=np.float32))
    assert uh.shape == (IN_NODES * OUT_NODES, F_SIZE), uh.shape
    uh = uh.reshape(IN_NODES, ROW)
    return [
        {"uh": np.ascontiguousarray(uh[k * I_LOC:(k + 1) * I_LOC])}
        for k in range(CORES)
    ]


def run(u_hat, routing_num, trace=False):
    nc = _get_nc(routing_num)
    in_maps = _shard(u_hat)
    res = bass_utils.run_bass_kernel_spmd(
        nc, in_maps, core_ids=list(range(CORES)), trace=trace)
    return res


def kernel(u_hat, routing_num):
    res = run(u_hat, routing_num, trace=False)
    return np.asarray(res.results[0]["v_out"], dtype=np.float32)



# revision 11
# speedup vs baseline: 1.5430x; 1.5430x over previous
"""DGL capsule routing layer on 8 trn2 NeuronCores (Bass/Tile), v2.

Math per pass t (b is linear in uh, so b_t = uh . w_t with w_t = cumsum v):
    b[i,o] = sum_f uh[i,o,f] * w[o,f]
    e = exp(b - 3); den[i] = sum_o e  (AllReduce over o-shards)
    chat[i,o] = e * (1/den)
    s[o,f] = sum_i chat[i,o] * uh[i,o,f]; v = squash(s); w += v
Pass 1 has chat uniform = 1/OUT.

Sharding: OUT_NODES split across 8 cores (128 local o per core). Each core
holds its full uh shard [4096 i, 128 o, 16 f] in SBUF as fp16 (host converts,
upload not counted in HW time). Per-core per pass:
  sweep1 (passes>=2): per block of 128 i: tm = uh*w_rep (DVE/GpSimd TT fp16),
    b = reduce_f (DVE, f32), e = exp(b-3) on ACT with fused den accum.
    den partials AllReduce'd in two halves so the first half's matmuls
    overlap the second half's sweep1.
  sweep2: chat_blk = e_blk * rinv (DVE tensor_scalar, fp16) becomes the PE
    STATIONARY [128 i, 128 o]; moving = uh block [128, 2048] -> psum
    [128 o, 2048 (o,f)] accumulated over all 32 blocks. The wanted s[o,f]
    is the "diagonal" ps[o, 16o+f], extracted via a DRAM round-trip with a
    stride-2080 read AP. squash on [128 o, 16] layout, w broadcast via DRAM.
Output: core c returns v slice [128, 16]; host concatenates.
"""

import numpy as np
from contextlib import ExitStack

import concourse.bass as bass
import concourse.mybir as mybir
import concourse.tile as tile
from concourse import bacc
from concourse import bass_utils

F32 = mybir.dt.float32
F16 = mybir.dt.float16
AX = mybir.AxisListType
AF = mybir.ActivationFunctionType

IN_NODES, OUT_NODES, F_SIZE = 4096, 1024, 16
CORES = 8
O_LOC = OUT_NODES // CORES         # 128 local out-nodes per core
P = 128
NBLK = IN_NODES // P               # 32 i-blocks per core
RB = O_LOC * F_SIZE                # 2048 elems per i-row (128 o x 16 f)
NST = 8                            # supertiles for sweep1
STB = NBLK // NST                  # 4 blocks per supertile
GPS_ST0 = 3                        # supertiles [GPS_ST0, NST) do tm on GpSimd
EXP_BIAS = -3.0                    # exp(b-3): keeps e in fp16 range
HALF = NBLK // 2

# DRAM scratch for the diagonal extract: write rows at stride 2064,
# read the diagonal (o, 16o+f) at stride 2080.
WSTR = RB + F_SIZE                 # 2064
DSTR = RB + 2 * F_SIZE             # 2080
SDUMP = P * DSTR                   # 266240 floats


def _sweep2_half(nc, small, s_ps, uh_sb, e_all, rinv_all, den_out, h):
    """chat = e * (1/den) on ACT only (keeps DVE/GpSimd free for sweep1 of
    the other half); PE accumulates s into psum across all blocks."""
    dsum = small.tile([P, HALF], F32, tag=f"dsum{h}")
    nc.sync.dma_start(dsum, den_out[h].rearrange("(p q) -> p q", p=P))
    lnd = small.tile([P, HALF], F32, tag=f"lnd{h}")
    nc.scalar.activation(lnd, dsum, AF.Ln)
    nc.scalar.activation(
        rinv_all[:, h * HALF:(h + 1) * HALF], lnd, AF.Exp, scale=-1.0)
    for k in range(HALF):
        blk = h * HALF + k
        ch = small.tile([P, P], F16, tag="ch", bufs=4)
        nc.scalar.mul(ch, e_all[:, blk * P:(blk + 1) * P],
                      rinv_all[:, blk:blk + 1])
        for c in range(RB // 512):
            nc.tensor.matmul(
                s_ps[:, c * 512:(c + 1) * 512],
                ch,
                uh_sb[:, blk * RB + c * 512:blk * RB + (c + 1) * 512],
                start=(blk == 0), stop=(blk == NBLK - 1),
                skip_group_check=True)


def _squash_and_w(nc, small, w_rep, s_sb, w_acc, t, R, v_out, dram):
    """s_sb [128 o, 16] f32 -> v; if not last pass, w_acc += v and broadcast
    w to w_rep [128, 2048] fp16 via DRAM."""
    ssq = small.tile([P, F_SIZE], F32, tag="ssq")
    nc.vector.tensor_mul(ssq, s_sb, s_sb)
    sq = small.tile([P, 1], F32, tag="sq")
    nc.vector.reduce_sum(sq, ssq, axis=AX.X)
    # y = sqrt(sq) via exp(0.5 ln sq) + one Newton step
    lnq = small.tile([P, 1], F32, tag="lnq")
    nc.scalar.activation(lnq, sq, AF.Ln)
    y = small.tile([P, 1], F32, tag="y")
    nc.scalar.activation(y, lnq, AF.Exp, scale=0.5)
    ry = small.tile([P, 1], F32, tag="ry")
    nc.vector.reciprocal(ry, y)
    t1 = small.tile([P, 1], F32, tag="t1")
    nc.vector.tensor_mul(t1, sq, ry)
    nc.vector.tensor_add(t1, t1, y)
    nc.vector.tensor_scalar_mul(t1, t1, 0.5)
    d1 = small.tile([P, 1], F32, tag="d1")
    nc.vector.tensor_scalar_add(d1, sq, 1.0)
    rd = small.tile([P, 1], F32, tag="rd")
    nc.vector.reciprocal(rd, d1)
    sc = small.tile([P, 1], F32, tag="sc")
    nc.vector.tensor_mul(sc, t1, rd)
    v_sb = small.tile([P, F_SIZE], F32, tag="v_sb")
    nc.vector.tensor_scalar_mul(v_sb, s_sb, sc)
    if t == R:
        nc.sync.dma_start(v_out, v_sb)
        return
    if t == 1:
        nc.scalar.copy(w_acc, v_sb)
    else:
        nc.vector.tensor_add(w_acc, w_acc, v_sb)
    # broadcast w_acc [128 o, 16] -> w_rep [128 i-part, 2048] fp16 via DRAM
    w_bf = small.tile([P, F_SIZE], F16, tag="w_bf")
    nc.vector.tensor_copy(w_bf, w_acc)
    w_flat = dram.tile([RB], F16, tag="w_flat")
    nc.sync.dma_start(w_flat.rearrange("(p q) -> p q", p=P), w_bf)
    nc.sync.dma_start(
        w_rep, w_flat.unsqueeze(0).broadcast_to([P, RB]))


def _body(nc, tc, uh_d, v_out, R, rg):
    with ExitStack() as ctx:
        persist = ctx.enter_context(tc.tile_pool(name="persist", bufs=1))
        work = ctx.enter_context(tc.tile_pool(name="work", bufs=2))
        small = ctx.enter_context(tc.tile_pool(name="small", bufs=2))
        pspool = ctx.enter_context(tc.tile_pool(name="pspool", bufs=1, space="PSUM"))
        dram = ctx.enter_context(tc.tile_pool(name="dram", bufs=2, space="DRAM"))

        uh_sb = persist.tile([P, NBLK * RB], F16, name="uh_sb")
        w_rep = persist.tile([P, RB], F16, name="w_rep")
        c0 = persist.tile([P, P], F16, name="c0")
        nc.vector.memset(c0, 1.0 / OUT_NODES)
        e_all = persist.tile([P, NBLK * P], F16, name="e_all")
        den_all = persist.tile([P, NBLK], F32, name="den_all")
        rinv_all = persist.tile([P, NBLK], F32, name="rinv_all")
        w_acc = persist.tile([P, F_SIZE], F32, name="w_acc")
        ebias = persist.tile([P, 1], F32, name="ebias")
        nc.vector.memset(ebias, EXP_BIAS)

        # upload the uh shard once (fp16 from host)
        for b in range(NBLK):
            nc.sync.dma_start(uh_sb[:, b * RB:(b + 1) * RB], uh_d[b])

        for t in range(1, R + 1):
            s_ps = pspool.tile([P, RB], F32, tag="s_ps")
            if t > 1:
                # ---- sweep1: b, e, den partials; sweep2 per AR-half ----
                den_dr, den_out = [], []
                for h in range(2):
                    d_in = dram.tile([P * HALF], F32, tag=f"den{h}")
                    d_out = dram.tile([P * HALF], F32, tag=f"deno{h}")
                    den_dr.append(d_in)
                    den_out.append(d_out)
                for st in range(NST):
                    g0 = st * STB * RB
                    sz = STB * RB
                    tm = work.tile([P, sz], F16, tag="tm")
                    eng = nc.gpsimd if st >= GPS_ST0 else nc.vector
                    eng.tensor_mul(
                        tm.rearrange("p (k r) -> p k r", r=RB),
                        uh_sb[:, g0:g0 + sz].rearrange(
                            "p (k r) -> p k r", r=RB),
                        w_rep[:, None, :].broadcast_to([P, STB, RB]),
                    )
                    b_sb = work.tile([P, STB * O_LOC], F32, tag="b_sb")
                    nc.vector.reduce_sum(
                        b_sb, tm.rearrange("p (o f) -> p o f", f=F_SIZE),
                        axis=AX.X)
                    for k in range(STB):
                        blk = st * STB + k
                        nc.scalar.activation(
                            e_all[:, blk * P:(blk + 1) * P],
                            b_sb[:, k * O_LOC:(k + 1) * O_LOC],
                            AF.Exp, bias=ebias,
                            accum_out=den_all[:, blk:blk + 1])
                    if (st + 1) * STB == HALF:
                        nc.sync.dma_start(
                            den_dr[0].rearrange("(p q) -> p q", p=P),
                            den_all[:, :HALF])
                        nc.gpsimd.collective_compute(
                            "AllReduce", mybir.AluOpType.add,
                            replica_groups=rg,
                            ins=[den_dr[0].opt()], outs=[den_out[0].opt()])
                        _sweep2_half(nc, small, s_ps, uh_sb, e_all,
                                     rinv_all, den_out, 0)
                nc.sync.dma_start(
                    den_dr[1].rearrange("(p q) -> p q", p=P),
                    den_all[:, HALF:])
                nc.gpsimd.collective_compute(
                    "AllReduce", mybir.AluOpType.add, replica_groups=rg,
                    ins=[den_dr[1].opt()], outs=[den_out[1].opt()])
                _sweep2_half(nc, small, s_ps, uh_sb, e_all,
                             rinv_all, den_out, 1)
            else:
                # pass 1: chat uniform = 1/OUT -> pure PE pass
                for blk in range(NBLK):
                    for c in range(RB // 512):
                        nc.tensor.matmul(
                            s_ps[:, c * 512:(c + 1) * 512],
                            c0,
                            uh_sb[:, blk * RB + c * 512:
                                  blk * RB + (c + 1) * 512],
                            start=(blk == 0), stop=(blk == NBLK - 1),
                            skip_group_check=True)
            # ---- diagonal extract: s[o,f] = ps[o, 16o+f] ----
            s_flat = work.tile([P, RB], F32, tag="s_flat")
            nc.scalar.copy(s_flat, s_ps)
            sdump = dram.tile([SDUMP], F32, tag="sdump")
            wview = sdump[0:P * WSTR].rearrange("(p q) -> p q", q=WSTR)
            nc.sync.dma_start(wview[:, 0:RB], s_flat)
            s_sb = small.tile([P, F_SIZE], F32, tag="s_sb")
            dview = sdump.rearrange("(p q) -> p q", q=DSTR)
            nc.sync.dma_start(s_sb, dview[:, 0:F_SIZE])
            _squash_and_w(nc, small, w_rep, s_sb, w_acc, t, R, v_out, dram)


def _build(routing_num: int):
    R = int(routing_num)
    assert R >= 1
    nc = bacc.Bacc(
        "TRN2", target_bir_lowering=False, debug=False, num_devices=CORES)
    uh = nc.dram_tensor("uh", [NBLK, P, RB], F16, kind="ExternalInput")
    v_out = nc.dram_tensor("v_out", [P, F_SIZE], F32, kind="ExternalOutput")
    rg = [list(range(CORES))]
    with tile.TileContext(nc) as tc:
        _body(nc, tc, uh.ap(), v_out.ap(), R, rg)
    nc.compile()
    return nc


_CACHE: dict = {}


def _get_nc(routing_num: int):
    R = int(routing_num)
    if R not in _CACHE:
        _CACHE[R] = _build(R)
    return _CACHE[R]


def _shard(u_hat: np.ndarray):
    uh = np.asarray(u_hat, dtype=np.float32)
    assert uh.shape == (IN_NODES * OUT_NODES, F_SIZE), uh.shape
    uh = uh.reshape(IN_NODES, OUT_NODES, F_SIZE)
    maps = []
    for c in range(CORES):
        sh = uh[:, c * O_LOC:(c + 1) * O_LOC, :].astype(np.float16)
        maps.append({"uh": np.ascontiguousarray(
            sh.reshape(NBLK, P, RB))})
    return maps


def run(u_hat, routing_num, trace=False):
    nc = _get_nc(routing_num)
    in_maps = _shard(u_hat)
    res = bass_utils.run_bass_kernel_spmd(
        nc, in_maps, core_ids=list(range(CORES)), trace=trace)
    return res


def kernel(u_hat, routing_num):
    res = run(u_hat, routing_num, trace=False)
    out = np.concatenate(
        [np.asarray(res.results[c]["v_out"], dtype=np.float32)
         for c in range(CORES)], axis=0)
    return out


# revision 15
# speedup vs baseline: 1.6740x; 1.0849x over previous
"""DGL capsule routing layer on 8 trn2 NeuronCores (Bass/Tile), v2.

Math per pass t (b is linear in uh, so b_t = uh . w_t with w_t = cumsum v):
    b[i,o] = sum_f uh[i,o,f] * w[o,f]
    e = exp(b - 3); den[i] = sum_o e  (AllReduce over o-shards)
    chat[i,o] = e * (1/den)
    s[o,f] = sum_i chat[i,o] * uh[i,o,f]; v = squash(s); w += v
Pass 1 has chat uniform = 1/OUT.

Sharding: OUT_NODES split across 8 cores (128 local o per core). Each core
holds its full uh shard [4096 i, 128 o, 16 f] in SBUF as fp16 (host converts,
upload not counted in HW time). Per-core per pass:
  sweep1 (passes>=2): per block of 128 i: tm = uh*w_rep (DVE/GpSimd TT fp16),
    b = reduce_f (DVE, f32), e = exp(b-3) on ACT with fused den accum.
    den partials AllReduce'd in two halves so the first half's matmuls
    overlap the second half's sweep1.
  sweep2: chat_blk = e_blk * rinv (DVE tensor_scalar, fp16) becomes the PE
    STATIONARY [128 i, 128 o]; moving = uh block [128, 2048] -> psum
    [128 o, 2048 (o,f)] accumulated over all 32 blocks. The wanted s[o,f]
    is the "diagonal" ps[o, 16o+f], extracted via a DRAM round-trip with a
    stride-2080 read AP. squash on [128 o, 16] layout, w broadcast via DRAM.
Output: core c returns v slice [128, 16]; host concatenates.
"""

import numpy as np
from contextlib import ExitStack

import concourse.bass as bass
import concourse.mybir as mybir
import concourse.tile as tile
from concourse import bacc
from concourse import bass_utils

F32 = mybir.dt.float32
F16 = mybir.dt.float16
AX = mybir.AxisListType
AF = mybir.ActivationFunctionType

IN_NODES, OUT_NODES, F_SIZE = 4096, 1024, 16
CORES = 8
O_LOC = OUT_NODES // CORES         # 128 local out-nodes per core
P = 128
NBLK = IN_NODES // P               # 32 i-blocks per core
RB = O_LOC * F_SIZE                # 2048 elems per i-row (128 o x 16 f)
NST = 8                            # supertiles for sweep1
STB = NBLK // NST                  # 4 blocks per supertile
GPS_ST0 = 4                        # supertiles [GPS_ST0, NST) do tm on GpSimd
EXP_BIAS = -3.0                    # exp(b-3): keeps e in fp16 range
MMW = 512                          # matmul moving width (one psum bank)

# DRAM scratch for the diagonal extract: write rows at stride 2064,
# read the diagonal (o, 16o+f) at stride 2080.
WSTR = RB + F_SIZE                 # 2064
DSTR = RB + 2 * F_SIZE             # 2080
SDUMP = P * DSTR                   # 266240 floats


def _sweep2(nc, small, s_ps, uh_sb, e_all, rinv_all, den_out):
    """chat = e * (1/den); rinv on DVE (idle post-sweep1), chat on ACT,
    PE accumulates s into psum across all 32 blocks."""
    dsum = small.tile([P, NBLK], F32, tag="dsum")
    nc.sync.dma_start(dsum, den_out.rearrange("(p q) -> p q", p=P))
    nc.vector.reciprocal(rinv_all, dsum)
    for blk in range(NBLK):
        ch = small.tile([P, P], F16, tag="ch", bufs=4)
        nc.scalar.mul(ch, e_all[:, blk * P:(blk + 1) * P],
                      rinv_all[:, blk:blk + 1])
        for c in range(RB // MMW):
            nc.tensor.matmul(
                s_ps[:, c * MMW:(c + 1) * MMW],
                ch,
                uh_sb[:, blk * RB + c * MMW:blk * RB + (c + 1) * MMW],
                start=(blk == 0), stop=(blk == NBLK - 1),
                skip_group_check=True)


def _squash_and_w(nc, small, w_rep, s_sb, w_acc, t, R, v_out, dram):
    """s_sb [128 o, 16] f32 -> v; if not last pass, w_acc += v and broadcast
    w to w_rep [128, 2048] fp16 via DRAM."""
    ssq = small.tile([P, F_SIZE], F32, tag="ssq")
    nc.vector.tensor_mul(ssq, s_sb, s_sb)
    sq = small.tile([P, 1], F32, tag="sq")
    nc.vector.reduce_sum(sq, ssq, axis=AX.X)
    # y = sqrt(sq) via exp(0.5 ln sq) + one Newton step
    lnq = small.tile([P, 1], F32, tag="lnq")
    nc.scalar.activation(lnq, sq, AF.Ln)
    y = small.tile([P, 1], F32, tag="y")
    nc.scalar.activation(y, lnq, AF.Exp, scale=0.5)
    ry = small.tile([P, 1], F32, tag="ry")
    nc.vector.reciprocal(ry, y)
    t1 = small.tile([P, 1], F32, tag="t1")
    nc.vector.tensor_mul(t1, sq, ry)
    nc.vector.tensor_add(t1, t1, y)
    nc.vector.tensor_scalar_mul(t1, t1, 0.5)
    d1 = small.tile([P, 1], F32, tag="d1")
    nc.vector.tensor_scalar_add(d1, sq, 1.0)
    rd = small.tile([P, 1], F32, tag="rd")
    nc.vector.reciprocal(rd, d1)
    sc = small.tile([P, 1], F32, tag="sc")
    nc.vector.tensor_mul(sc, t1, rd)
    v_sb = small.tile([P, F_SIZE], F32, tag="v_sb")
    nc.vector.tensor_scalar_mul(v_sb, s_sb, sc)
    if t == R:
        nc.sync.dma_start(v_out, v_sb)
        return
    if t == 1:
        nc.scalar.copy(w_acc, v_sb)
    else:
        nc.vector.tensor_add(w_acc, w_acc, v_sb)
    # broadcast w_acc [128 o, 16] -> w_rep [128 i-part, 2048] fp16 via DRAM
    w_bf = small.tile([P, F_SIZE], F16, tag="w_bf")
    nc.vector.tensor_copy(w_bf, w_acc)
    w_flat = dram.tile([RB], F16, tag="w_flat")
    nc.sync.dma_start(w_flat.rearrange("(p q) -> p q", p=P), w_bf)
    nc.sync.dma_start(
        w_rep, w_flat.unsqueeze(0).broadcast_to([P, RB]))


def _body(nc, tc, uh_d, v_out, R, rg):
    with ExitStack() as ctx:
        persist = ctx.enter_context(tc.tile_pool(name="persist", bufs=1))
        work = ctx.enter_context(tc.tile_pool(name="work", bufs=2))
        small = ctx.enter_context(tc.tile_pool(name="small", bufs=2))
        pspool = ctx.enter_context(tc.tile_pool(name="pspool", bufs=1, space="PSUM"))
        dram = ctx.enter_context(tc.tile_pool(name="dram", bufs=2, space="DRAM"))

        uh_sb = persist.tile([P, NBLK * RB], F16, name="uh_sb")
        w_rep = persist.tile([P, RB], F16, name="w_rep")
        c0 = persist.tile([P, P], F16, name="c0")
        nc.vector.memset(c0, 1.0 / OUT_NODES)
        e_all = persist.tile([P, NBLK * P], F16, name="e_all")
        den_all = persist.tile([P, NBLK], F32, name="den_all")
        rinv_all = persist.tile([P, NBLK], F32, name="rinv_all")
        w_acc = persist.tile([P, F_SIZE], F32, name="w_acc")
        ebias = persist.tile([P, 1], F32, name="ebias")
        nc.vector.memset(ebias, EXP_BIAS)

        # upload the uh shard once (fp16 from host)
        for b in range(NBLK):
            nc.sync.dma_start(uh_sb[:, b * RB:(b + 1) * RB], uh_d[b])

        for t in range(1, R + 1):
            s_ps = pspool.tile([P, RB], F32, tag="s_ps")
            if t > 1:
                # ---- sweep1: b, e, den partials; one AR at the end ----
                den_dr = dram.tile([P * NBLK], F32, tag="den_dr")
                den_out = dram.tile([P * NBLK], F32, tag="den_out")
                for st in range(NST):
                    g0 = st * STB * RB
                    sz = STB * RB
                    tm = work.tile([P, sz], F16, tag="tm")
                    eng = nc.gpsimd if st >= GPS_ST0 else nc.vector
                    eng.tensor_mul(
                        tm.rearrange("p (k r) -> p k r", r=RB),
                        uh_sb[:, g0:g0 + sz].rearrange(
                            "p (k r) -> p k r", r=RB),
                        w_rep[:, None, :].broadcast_to([P, STB, RB]),
                    )
                    b_sb = work.tile([P, STB * O_LOC], F32, tag="b_sb")
                    nc.vector.reduce_sum(
                        b_sb, tm.rearrange("p (o f) -> p o f", f=F_SIZE),
                        axis=AX.X)
                    for k in range(STB):
                        blk = st * STB + k
                        nc.scalar.activation(
                            e_all[:, blk * P:(blk + 1) * P],
                            b_sb[:, k * O_LOC:(k + 1) * O_LOC],
                            AF.Exp, bias=ebias,
                            accum_out=den_all[:, blk:blk + 1])
                nc.sync.dma_start(
                    den_dr.rearrange("(p q) -> p q", p=P), den_all)
                nc.gpsimd.collective_compute(
                    "AllReduce", mybir.AluOpType.add, replica_groups=rg,
                    ins=[den_dr.opt()], outs=[den_out.opt()])
                _sweep2(nc, small, s_ps, uh_sb, e_all, rinv_all, den_out)
            else:
                # pass 1: chat uniform = 1/OUT -> pure PE pass
                for blk in range(NBLK):
                    for c in range(RB // MMW):
                        nc.tensor.matmul(
                            s_ps[:, c * MMW:(c + 1) * MMW],
                            c0,
                            uh_sb[:, blk * RB + c * MMW:
                                  blk * RB + (c + 1) * MMW],
                            start=(blk == 0), stop=(blk == NBLK - 1),
                            skip_group_check=True)
            # ---- diagonal extract: s[o,f] = ps[o, 16o+f] ----
            s_flat = work.tile([P, RB], F32, tag="s_flat")
            nc.scalar.copy(s_flat, s_ps)
            sdump = dram.tile([SDUMP], F32, tag="sdump")
            wview = sdump[0:P * WSTR].rearrange("(p q) -> p q", q=WSTR)
            nc.sync.dma_start(wview[:, 0:RB], s_flat)
            s_sb = small.tile([P, F_SIZE], F32, tag="s_sb")
            dview = sdump.rearrange("(p q) -> p q", q=DSTR)
            nc.sync.dma_start(s_sb, dview[:, 0:F_SIZE])
            _squash_and_w(nc, small, w_rep, s_sb, w_acc, t, R, v_out, dram)


def _build(routing_num: int):
    R = int(routing_num)
    assert R >= 1
    nc = bacc.Bacc(
        "TRN2", target_bir_lowering=False, debug=False, num_devices=CORES)
    uh = nc.dram_tensor("uh", [NBLK, P, RB], F16, kind="ExternalInput")
    v_out = nc.dram_tensor("v_out", [P, F_SIZE], F32, kind="ExternalOutput")
    rg = [list(range(CORES))]
    with tile.TileContext(nc) as tc:
        _body(nc, tc, uh.ap(), v_out.ap(), R, rg)
    nc.compile()
    return nc


_CACHE: dict = {}


def _get_nc(routing_num: int):
    R = int(routing_num)
    if R not in _CACHE:
        _CACHE[R] = _build(R)
    return _CACHE[R]


def _shard(u_hat: np.ndarray):
    uh = np.asarray(u_hat, dtype=np.float32)
    assert uh.shape == (IN_NODES * OUT_NODES, F_SIZE), uh.shape
    uh = uh.reshape(IN_NODES, OUT_NODES, F_SIZE)
    maps = []
    for c in range(CORES):
        sh = uh[:, c * O_LOC:(c + 1) * O_LOC, :].astype(np.float16)
        maps.append({"uh": np.ascontiguousarray(
            sh.reshape(NBLK, P, RB))})
    return maps


def run(u_hat, routing_num, trace=False):
    nc = _get_nc(routing_num)
    in_maps = _shard(u_hat)
    res = bass_utils.run_bass_kernel_spmd(
        nc, in_maps, core_ids=list(range(CORES)), trace=trace)
    return res


def kernel(u_hat, routing_num):
    res = run(u_hat, routing_num, trace=False)
    out = np.concatenate(
        [np.asarray(res.results[c]["v_out"], dtype=np.float32)
         for c in range(CORES)], axis=0)
    return out


# revision 21
# speedup vs baseline: 1.7071x; 1.0198x over previous
"""DGL capsule routing layer on 8 trn2 NeuronCores (Bass/Tile), v2.

Math per pass t (b is linear in uh, so b_t = uh . w_t with w_t = cumsum v):
    b[i,o] = sum_f uh[i,o,f] * w[o,f]
    e = exp(b - 3); den[i] = sum_o e  (AllReduce over o-shards)
    chat[i,o] = e * (1/den)
    s[o,f] = sum_i chat[i,o] * uh[i,o,f]; v = squash(s); w += v
Pass 1 has chat uniform = 1/OUT.

Sharding: OUT_NODES split across 8 cores (128 local o per core). Each core
holds its full uh shard [4096 i, 128 o, 16 f] in SBUF as fp16 (host converts,
upload not counted in HW time). Per-core per pass:
  sweep1 (passes>=2): per block of 128 i: tm = uh*w_rep (DVE/GpSimd TT fp16),
    b = reduce_f (DVE, f32), e = exp(b-3) on ACT with fused den accum.
    den partials AllReduce'd in two halves so the first half's matmuls
    overlap the second half's sweep1.
  sweep2: chat_blk = e_blk * rinv (DVE tensor_scalar, fp16) becomes the PE
    STATIONARY [128 i, 128 o]; moving = uh block [128, 2048] -> psum
    [128 o, 2048 (o,f)] accumulated over all 32 blocks. The wanted s[o,f]
    is the "diagonal" ps[o, 16o+f], extracted via a DRAM round-trip with a
    stride-2080 read AP. squash on [128 o, 16] layout, w broadcast via DRAM.
Output: core c returns v slice [128, 16]; host concatenates.
"""

import numpy as np
from contextlib import ExitStack

import concourse.bass as bass
import concourse.mybir as mybir
import concourse.tile as tile
from concourse import bacc
from concourse import bass_utils

F32 = mybir.dt.float32
F16 = mybir.dt.float16
AX = mybir.AxisListType
AF = mybir.ActivationFunctionType

IN_NODES, OUT_NODES, F_SIZE = 4096, 1024, 16
CORES = 8
O_LOC = OUT_NODES // CORES         # 128 local out-nodes per core
P = 128
NBLK = IN_NODES // P               # 32 i-blocks per core
RB = O_LOC * F_SIZE                # 2048 elems per i-row (128 o x 16 f)
DVE_BLKS = 16                      # blocks 0..15: tm on DVE (2-block tiles)
DVE_STB = 2
GPS_STB = 4                        # blocks 16..31: tm on GpSimd (4-block tiles)
EXP_BIAS = -3.0                    # exp(b-3): keeps e in fp16 range
MMW = 512                          # matmul moving width (one psum bank)
U32 = mybir.dt.uint32
ALU = mybir.AluOpType


def _mm(nc, out, lhsT, rhs, start, stop, load):
    r = nc.tensor.matmul(out, lhsT, rhs, start=start, stop=stop,
                         skip_group_check=True)
    if not load:
        # stationary identical to the previous matmul's: skip the LDWEIGHTS
        r.ins.ldweights = False
    return r

# DRAM scratch for the diagonal extract: write rows at stride 2064,
# read the diagonal (o, 16o+f) at stride 2080.
WSTR = RB + F_SIZE                 # 2064
DSTR = RB + 2 * F_SIZE             # 2080
SDUMP = P * DSTR                   # 266240 floats


def _sweep2(nc, small, s_ps, uh_sb, e_all, rinv_all, den_out):
    """chat = e * (1/den); rinv on DVE (idle post-sweep1), chat on ACT,
    PE accumulates s into psum across all 32 blocks."""
    dsum = small.tile([P, NBLK], F32, tag="dsum")
    nc.sync.dma_start(dsum, den_out.rearrange("(p q) -> p q", p=P))
    nc.vector.reciprocal(rinv_all, dsum)
    for blk in range(NBLK):
        ch = small.tile([P, P], F16, tag="ch", bufs=4)
        nc.scalar.mul(ch, e_all[:, blk * P:(blk + 1) * P],
                      rinv_all[:, blk:blk + 1])
        for c in range(RB // MMW):
            _mm(nc, s_ps[:, c * MMW:(c + 1) * MMW],
                ch,
                uh_sb[:, blk * RB + c * MMW:blk * RB + (c + 1) * MMW],
                start=(blk == 0), stop=(blk == NBLK - 1), load=(c == 0))


def _squash_and_w(nc, small, w_rep, s_sb, w_acc, t, R, v_out, dram):
    """s_sb [128 o, 16] f32 -> v; if not last pass, w_acc += v and broadcast
    w to w_rep [128, 2048] fp16 via DRAM."""
    ssq = small.tile([P, F_SIZE], F32, tag="ssq")
    nc.vector.tensor_mul(ssq, s_sb, s_sb)
    sq = small.tile([P, 1], F32, tag="sq")
    nc.vector.reduce_sum(sq, ssq, axis=AX.X)
    # y = sqrt(sq) via exp(0.5 ln sq) + one Newton step
    lnq = small.tile([P, 1], F32, tag="lnq")
    nc.scalar.activation(lnq, sq, AF.Ln)
    y = small.tile([P, 1], F32, tag="y")
    nc.scalar.activation(y, lnq, AF.Exp, scale=0.5)
    ry = small.tile([P, 1], F32, tag="ry")
    nc.vector.reciprocal(ry, y)
    t1 = small.tile([P, 1], F32, tag="t1")
    nc.vector.tensor_mul(t1, sq, ry)
    nc.vector.tensor_add(t1, t1, y)
    nc.vector.tensor_scalar_mul(t1, t1, 0.5)
    d1 = small.tile([P, 1], F32, tag="d1")
    nc.vector.tensor_scalar_add(d1, sq, 1.0)
    rd = small.tile([P, 1], F32, tag="rd")
    nc.vector.reciprocal(rd, d1)
    sc = small.tile([P, 1], F32, tag="sc")
    nc.vector.tensor_mul(sc, t1, rd)
    v_sb = small.tile([P, F_SIZE], F32, tag="v_sb")
    nc.vector.tensor_scalar_mul(v_sb, s_sb, sc)
    if t == R:
        nc.sync.dma_start(v_out, v_sb)
        return
    if t == 1:
        nc.scalar.copy(w_acc, v_sb)
    else:
        nc.vector.tensor_add(w_acc, w_acc, v_sb)
    # broadcast w_acc [128 o, 16] -> w_rep [128 i-part, 2048] fp16 via DRAM
    w_bf = small.tile([P, F_SIZE], F16, tag="w_bf")
    nc.vector.tensor_copy(w_bf, w_acc)
    w_flat = dram.tile([RB], F16, tag="w_flat")
    nc.sync.dma_start(w_flat.rearrange("(p q) -> p q", p=P), w_bf)
    nc.sync.dma_start(
        w_rep, w_flat.unsqueeze(0).broadcast_to([P, RB]))


def _body(nc, tc, uh_d, v_out, R, rg):
    with ExitStack() as ctx:
        persist = ctx.enter_context(tc.tile_pool(name="persist", bufs=1))
        work = ctx.enter_context(tc.tile_pool(name="work", bufs=2))
        small = ctx.enter_context(tc.tile_pool(name="small", bufs=2))
        pspool = ctx.enter_context(tc.tile_pool(name="pspool", bufs=1, space="PSUM"))
        dram = ctx.enter_context(tc.tile_pool(name="dram", bufs=2, space="DRAM"))

        uh_sb = persist.tile([P, NBLK * RB], F16, name="uh_sb")
        w_rep = persist.tile([P, RB], F16, name="w_rep")
        c0 = persist.tile([P, P], F16, name="c0")
        nc.vector.memset(c0, 1.0 / OUT_NODES)
        e_all = persist.tile([P, NBLK * P], F16, name="e_all")
        den_all = persist.tile([P, NBLK], F32, name="den_all")
        rinv_all = persist.tile([P, NBLK], F32, name="rinv_all")
        w_acc = persist.tile([P, F_SIZE], F32, name="w_acc")
        ebias = persist.tile([P, 1], F32, name="ebias")
        nc.vector.memset(ebias, EXP_BIAS)

        # upload the uh shard once (fp16 from host)
        for b in range(NBLK):
            nc.sync.dma_start(uh_sb[:, b * RB:(b + 1) * RB], uh_d[b])

        for t in range(1, R + 1):
            s_ps = pspool.tile([P, RB], F32, tag="s_ps")
            if t > 1:
                # ---- sweep1: b, e, den partials; one AR at the end ----
                den_dr = dram.tile([P * NBLK], F32, tag="den_dr")
                den_out = dram.tile([P * NBLK], F32, tag="den_out")
                # GpSimd tms first so they start at pass begin (own tag:
                # no WAR coupling with the DVE tiles)
                gps_tms = []
                for gst in range((NBLK - DVE_BLKS) // GPS_STB):
                    g0 = (DVE_BLKS + gst * GPS_STB) * RB
                    sz = GPS_STB * RB
                    tg = work.tile([P, sz], F16, tag="tm_gps")
                    nc.gpsimd.tensor_mul(
                        tg.rearrange("p (k r) -> p k r", r=RB),
                        uh_sb[:, g0:g0 + sz].rearrange(
                            "p (k r) -> p k r", r=RB),
                        w_rep[:, None, :].broadcast_to([P, GPS_STB, RB]),
                    )
                    gps_tms.append(tg)
                # DVE supertiles: tm + reduce + exp
                for st in range(DVE_BLKS // DVE_STB):
                    g0 = st * DVE_STB * RB
                    sz = DVE_STB * RB
                    tm = work.tile([P, sz], F16, tag="tm_dve")
                    nc.vector.tensor_mul(
                        tm.rearrange("p (k r) -> p k r", r=RB),
                        uh_sb[:, g0:g0 + sz].rearrange(
                            "p (k r) -> p k r", r=RB),
                        w_rep[:, None, :].broadcast_to([P, DVE_STB, RB]),
                    )
                    b_sb = work.tile([P, DVE_STB * O_LOC], F32, tag="b_dve")
                    nc.vector.reduce_sum(
                        b_sb, tm.rearrange("p (o f) -> p o f", f=F_SIZE),
                        axis=AX.X)
                    for k in range(DVE_STB):
                        blk = st * DVE_STB + k
                        nc.scalar.activation(
                            e_all[:, blk * P:(blk + 1) * P],
                            b_sb[:, k * O_LOC:(k + 1) * O_LOC],
                            AF.Exp, bias=ebias,
                            accum_out=den_all[:, blk:blk + 1])
                # reduces + exps for the GpSimd-produced tms
                for gst, tg in enumerate(gps_tms):
                    b_sb = work.tile([P, GPS_STB * O_LOC], F32, tag="b_gps")
                    nc.vector.reduce_sum(
                        b_sb, tg.rearrange("p (o f) -> p o f", f=F_SIZE),
                        axis=AX.X)
                    for k in range(GPS_STB):
                        blk = DVE_BLKS + gst * GPS_STB + k
                        nc.scalar.activation(
                            e_all[:, blk * P:(blk + 1) * P],
                            b_sb[:, k * O_LOC:(k + 1) * O_LOC],
                            AF.Exp, bias=ebias,
                            accum_out=den_all[:, blk:blk + 1])
                nc.sync.dma_start(
                    den_dr.rearrange("(p q) -> p q", p=P), den_all)
                nc.gpsimd.collective_compute(
                    "AllReduce", mybir.AluOpType.add, replica_groups=rg,
                    ins=[den_dr.opt()], outs=[den_out.opt()])
                _sweep2(nc, small, s_ps, uh_sb, e_all, rinv_all, den_out)
            else:
                # pass 1: chat uniform = 1/OUT -> pure PE pass
                for blk in range(NBLK):
                    for c in range(RB // MMW):
                        _mm(nc, s_ps[:, c * MMW:(c + 1) * MMW],
                            c0,
                            uh_sb[:, blk * RB + c * MMW:
                                  blk * RB + (c + 1) * MMW],
                            start=(blk == 0), stop=(blk == NBLK - 1),
                            load=(blk == 0 and c == 0))
            # ---- diagonal extract: s[o,f] = ps[o, 16o+f] ----
            s_flat = work.tile([P, RB], F32, tag="s_flat", bufs=1)
            nc.scalar.copy(s_flat, s_ps)
            sdump = dram.tile([SDUMP], F32, tag="sdump")
            wview = sdump[0:P * WSTR].rearrange("(p q) -> p q", q=WSTR)
            nc.sync.dma_start(wview[:, 0:RB], s_flat)
            s_sb = small.tile([P, F_SIZE], F32, tag="s_sb")
            dview = sdump.rearrange("(p q) -> p q", q=DSTR)
            nc.sync.dma_start(s_sb, dview[:, 0:F_SIZE])
            _squash_and_w(nc, small, w_rep, s_sb, w_acc, t, R, v_out, dram)


def _build(routing_num: int):
    R = int(routing_num)
    assert R >= 1
    nc = bacc.Bacc(
        "TRN2", target_bir_lowering=False, debug=False, num_devices=CORES)
    uh = nc.dram_tensor("uh", [NBLK, P, RB], F16, kind="ExternalInput")
    v_out = nc.dram_tensor("v_out", [P, F_SIZE], F32, kind="ExternalOutput")
    rg = [list(range(CORES))]
    with tile.TileContext(nc) as tc:
        _body(nc, tc, uh.ap(), v_out.ap(), R, rg)
    nc.compile()
    return nc


_CACHE: dict = {}


def _get_nc(routing_num: int):
    R = int(routing_num)
    if R not in _CACHE:
        _CACHE[R] = _build(R)
    return _CACHE[R]


def _shard(u_hat: np.ndarray):
    uh = np.asarray(u_hat, dtype=np.float32)
    assert uh.shape == (IN_NODES * OUT_NODES, F_SIZE), uh.shape
    uh = uh.reshape(IN_NODES, OUT_NODES, F_SIZE)
    maps = []
    for c in range(CORES):
        sh = uh[:, c * O_LOC:(c + 1) * O_LOC, :].astype(np.float16)
        maps.append({"uh": np.ascontiguousarray(
            sh.reshape(NBLK, P, RB))})
    return maps


def run(u_hat, routing_num, trace=False):
    nc = _get_nc(routing_num)
    in_maps = _shard(u_hat)
    res = bass_utils.run_bass_kernel_spmd(
        nc, in_maps, core_ids=list(range(CORES)), trace=trace)
    return res


def kernel(u_hat, routing_num):
    res = run(u_hat, routing_num, trace=False)
    out = np.concatenate(
        [np.asarray(res.results[c]["v_out"], dtype=np.float32)
         for c in range(CORES)], axis=0)
    return out
